# revision 37
# baseline (speedup 1.0000x reference)
"""Trainium2 Bass kernel for batched CRF forward algorithm (log-partition).

Reference: alpha_t[b,i] = logsumexp_j(alpha_{t-1}[b,j] + trans[i,j]) + feat_t[b,i]
           logZ[b] = logsumexp_i(alpha_{T-1}[b,i] + trans[STOP,i])

Exp-domain device recursion per step: X' = (W @ X) * F with
F = exp(feat + BIAS_C) precomputed on host (bf16).

Time is cut into S=24 forward segments run in lockstep (Birkhoff
contraction: a 1-step warm-up from a ones vector leaves ~e-4 direction
error; unknown scales cancel in telescoped bridge corrections computed
from per-slot column sums, all evaluated on-device by tiny ones/wstop
matmuls into one PSUM bank -- the only kernel output).

Per core: 3 chains x [96, 1024] state tiles (2 lanes of 48 tags x 8
column groups of 128 seqs = 16 slots/chain), R=12 rounds. Per round per
chain: 2 matmuls (one per PSUM bank), then the PSUM->SBUF move is split
to balance engines: DVE does a direct 1x TT on cols [0:A), ACT copies
cols [A:1024) to SBUF bf16 where DVE re-TTs them at 2x mode. Measured
steady-state: DVE ~= ACT ~= 790 ns per chain-step -> ~2.37 us/round.

Sharding: B=2048 over 8 cores (data parallel), no collectives.
"""

import numpy as np

B, T, K = 2048, 256, 48
NCORE = 8
START, STOP = 46, 47
BIAS_C = -4.33

C = 3                    # chains per core
PP = 2 * K               # 96 partitions (2 lanes)
GW = 128                 # slot (group) width in columns
NG = 8                   # column groups per chain
CT = NG * GW             # 1024 columns per chain tile
R = 12                   # rounds
S = 24                   # segments (= C * NG * 2 lanes / 2 blocks)
WARM = 1                 # warm-up rounds per non-exact segment
ASPL = 216               # DVE-direct columns; ACT copies the rest
NTILE = R * C            # 36 F tiles consumed in (r, c) order
CHUNKS = [1] * 12 + [2] * 12   # F tiles per DMA chunk
NWARM = 3                # dummy matmuls to release the PE HAM clock gate
NLHS = 7                 # sum-event lhsT pairs: M x3, S0, F x3
CW = PP + NLHS * 2 * 48 + 2 * GW  # consts: W2 | event lhsTs | seg0 init

# segment schedule (mirrors sim.py): junction deltas sum to T - R
_deltas = [R - WARM] * (S - 1)
_exc = (R - WARM) * (S - 1) - (T - R)
_i = 0
while _exc > 0:
    _t = min(_exc, R - WARM - 1)
    _deltas[_i] -= _t
    _exc -= _t
    _i += 1
STARTS = np.concatenate([[0], np.cumsum(_deltas)]).astype(int)
RSTARS = [d + WARM - 1 for d in _deltas]     # E_s snapshot round per junction
EVROUNDS = sorted(set(RSTARS))               # extra E-event rounds (2 and 11)
assert STARTS[-1] == T - R and sum(CHUNKS) == NTILE

_cache = {}


def _seg_of(c, l, q):
    return 8 * c + 4 * l + q // 2


def _slot_of(s, b):
    """segment s, block b -> (chain, lane, group)."""
    return s // 8, (s % 8) // 4, 2 * (s % 4) + b


def _build():
    import concourse.bass as bass
    import concourse.bacc as bacc
    import concourse.mybir as mybir
    from concourse import tile

    bf16 = mybir.dt.bfloat16
    f32 = mybir.dt.float32
    PSUM = bass.MemorySpace.PSUM

    nc = bacc.Bacc(None, target_bir_lowering=False)

    fdr = [nc.dram_tensor(f"feats{q}", [PP, n * CT], bf16,
                          kind="ExternalInput") for q, n in enumerate(CHUNKS)]
    cdr = nc.dram_tensor("consts", [PP, PP + 2 * GW], bf16,
                         kind="ExternalInput")
    edr = nc.dram_tensor("evconsts", [PP, NLHS * 2 * 48], bf16,
                         kind="ExternalInput")
    odr = nc.dram_tensor("sums", [48, 512], f32, kind="ExternalOutput")

    with tile.TileContext(nc) as tc:
        with (
            tc.tile_pool(name="const", bufs=1) as cpool,
            tc.tile_pool(name="fchunk", bufs=1) as fpool,
            tc.tile_pool(name="state", bufs=2) as spool,
            tc.tile_pool(name="ycp", bufs=1) as ypool,
            tc.tile_pool(name="ps", bufs=1, space=PSUM) as pspool,
        ):
            csb = cpool.tile([PP, PP + 2 * GW], bf16, name="consts",
                             tag="consts")
            esb = cpool.tile([PP, NLHS * 2 * 48], bf16, name="evconsts",
                             tag="evconsts")
            isb = cpool.tile([PP, C * CT], bf16, name="inits", tag="inits")
            osb = cpool.tile([48, 512], f32, name="osums", tag="osums")
            w2 = csb[:, 0:PP]

            def evlhs(pair, h):
                off = (2 * pair + h) * 48
                return esb[:, off:off + 48]

            # warm-up scratch: memset is fast and has no DMA dependency, so
            # the PE burst can start right after the runtime preamble
            wt = cpool.tile([PP, 512], bf16, name="wtile", tag="wtile")
            nc.gpsimd.memset(wt[:], 1.0)

            # dummy activation: pulls the ~1.3us ACT_TABLE_LOAD out of
            # the main loop (first scalar.copy otherwise pays it there)
            nc.scalar.copy(wt[0:1, 0:8], wt[0:1, 8:16])

            nc.sync.dma_start(csb[:], cdr[:])
            # inits built on-device: ones everywhere, one-hot START for
            # segment 0 (chain 0, lane 0, groups 0..1) copied from consts
            nc.gpsimd.memset(isb[:], 1.0)
            nc.vector.tensor_copy(isb[:, 0:2 * GW], csb[:, PP:PP + 2 * GW])

            fts = []
            cstart = np.cumsum([0] + CHUNKS)
            for q, n in enumerate(CHUNKS):
                ft = fpool.tile([PP, n * CT], bf16, name=f"f{q}", tag=f"f{q}")
                # all on the sync HWDGE ring: one queue sustains the full
                # DMA rate, and FIFO order matches consumption order.
                # (A second ring steals little extra bandwidth, and issues
                # on the scalar ring stall the ACT sequencer mid-loop.)
                nc.sync.dma_start(ft[:], fdr[q][:])
                if q == 0:
                    nc.sync.dma_start(esb[:], edr[:])
                fts.append(ft)

            def ftile(r, c):
                m = r * C + c
                q = int(np.searchsorted(cstart, m, side="right")) - 1
                off = (m - int(cstart[q])) * CT
                return fts[q][:, off:off + CT]

            psum = [pspool.tile([PP, CT], f32, name=f"p{c}", tag=f"p{c}")
                    for c in range(C)]
            sums = pspool.tile([48, 512], f32, name="sums", tag="sums")
            scr = pspool.tile([PP, 512], f32, name="scr", tag="scr")

            # back-to-back dummy matmuls during the DMA head keep the PE
            # HAM activity window busy so the clock gate opens (1.2->2.4
            # GHz) before the main loop starts
            for _ in range(NWARM):
                nc.tensor.matmul(scr[:], wt[:, 0:PP], wt[:],
                                 start=True, stop=True)
            ys = [ypool.tile([PP, CT - ASPL], bf16, name=f"y{c}", tag=f"y{c}")
                  for c in range(C)]

            xs = [isb[:, c * CT:(c + 1) * CT] for c in range(C)]

            # PE fillers: the HAM clock gate re-throttles on sub-us PE idle
            # gaps, so keep PE near-continuously busy in every round
            # (fillers run in otherwise-idle PE windows; in-order issue
            # delays a ready real matmul by at most one filler)
            def filler(n):
                for _ in range(n):
                    nc.tensor.matmul(scr[:], wt[:, 0:PP], wt[:],
                                     start=True, stop=True)

            for r in range(R):
                xn = [spool.tile([PP, CT], bf16, name=f"x{c}", tag=f"x{c}")[:]
                      for c in range(C)]
                for c in range(C):
                    nc.tensor.matmul(psum[c][:, 0:512], w2, xs[c][:, 0:512],
                                     start=True, stop=True)
                    nc.tensor.matmul(psum[c][:, 512:CT], w2, xs[c][:, 512:CT],
                                     start=True, stop=True)
                    filler(1 if c < 2 else 2)
                if r <= 1:
                    filler(2)
                if r == R - 1:
                    filler(3)   # bridge PE into the tail event matmuls
                # direct TTs first so DVE is not stalled behind the copies
                for c in range(C):
                    f = ftile(r, c)
                    nc.vector.tensor_mul(xn[c][:, 0:ASPL],
                                         psum[c][:, 0:ASPL], f[:, 0:ASPL])
                for c in range(C):
                    nc.scalar.copy(ys[c][:], psum[c][:, ASPL:CT])
                for c in range(C):
                    f = ftile(r, c)
                    nc.vector.tensor_mul(xn[c][:, ASPL:CT],
                                         ys[c][:], f[:, ASPL:CT])
                xs = xn

                # bridge sums: one accumulation group into sums[48,512].
                # Row selection is encoded in sparse [96,48] lhsTs:
                # rows 16c+{0:4}=M(r=0), {4:8}=E(r=R-1), {8:12}=E(r=2) on
                # chain 0 / wstop(r=R-1) on chain 2.  lhsT pairs: M_c=0..2,
                # S0=3 (chain0 r=2), F_c=4..6 (E final, +wstop on chain 2).
                for c in range(C):
                    evs = []
                    if r == WARM - 1:
                        evs.append(c)               # M_c
                    if r == RSTARS[0] and c == 0 and RSTARS[0] != R - 1:
                        evs.append(3)               # S0
                    if r == R - 1:
                        evs.append(4 + c)           # F_c
                    if r == R - 1:
                        filler(1)
                    for pair in evs:
                        for h in range(2):
                            first = (r == WARM - 1 and c == 0 and h == 0)
                            last = (r == R - 1 and c == C - 1 and h == 1)
                            nc.tensor.matmul(
                                sums[:, :], evlhs(pair, h),
                                xs[c][:, 512 * h:512 * (h + 1)],
                                start=first, stop=last,
                                skip_group_check=True)

            nc.scalar.copy(osb[:], sums[:])
            nc.sync.dma_start(odr[:], osb[:])

    nc.compile()
    return nc


def _pack_host(feats, transitions):
    """Host-side sharding/layout prep (numpy only)."""
    import ml_dtypes

    feats = np.asarray(feats, dtype=np.float32)
    trans = np.asarray(transitions, dtype=np.float64)

    F = np.exp(feats + BIAS_C).reshape(NCORE, 2, GW, T, K)

    # F tiles: arr[core, part, r, c, q, col]
    arr = np.empty((NCORE, PP, R, C, NG, GW), dtype=np.float32)
    rr = np.arange(R)
    for c in range(C):
        for l in range(2):
            for q in range(NG):
                s = _seg_of(c, l, q)
                b = q % 2
                ts = STARTS[s] + rr
                # F[:, b, :, ts, :] -> [core, col, R, K] -> [core, K, R, col]
                blk = F[:, b][:, :, ts, :].transpose(0, 3, 2, 1)
                arr[:, 48 * l:48 * (l + 1), :, c, q, :] = blk
    flat = np.ascontiguousarray(
        arr.reshape(NCORE, PP, NTILE * CT)).astype(ml_dtypes.bfloat16)
    cstart = np.cumsum([0] + CHUNKS)
    chunks = [np.ascontiguousarray(flat[:, :, cstart[q] * CT:cstart[q + 1] * CT])
              for q in range(len(CHUNKS))]

    W = np.exp(trans)
    consts = np.zeros((PP, PP + 2 * GW), dtype=np.float64)
    consts[:K, :K] = W.T
    consts[K:, K:PP] = W.T
    evconsts = np.zeros((PP, NLHS * 2 * 48), dtype=np.float64)
    wstop = np.exp(trans[STOP])
    # event lhsTs [96,48]: column j routes a lane-sum to output row j
    for pair in range(NLHS):
        for h in range(2):
            L = np.zeros((PP, 48), dtype=np.float64)
            if pair < 3:                      # M_c: rows 16c+2h+l
                c, base = pair, 0
            elif pair == 3:                   # S0: chain0 r=2, rows 8+2h+l
                c, base = 0, 8
            else:                             # F_c: rows 16c+4+2h+l
                c, base = pair - 4, 4
            for l in range(2):
                L[48 * l:48 * (l + 1), 16 * c + base + 2 * h + l] = 1.0
            if pair == 6:                     # chain2 final: + wstop rows
                L[K:, 16 * 2 + 8 + 2 * h + 1] = wstop
            evconsts[:, (2 * pair + h) * 48:(2 * pair + h + 1) * 48] = L
    consts[K:, PP:PP + 2 * GW] = 1.0          # seg0 init block: lane1 ones
    consts[START, PP:PP + 2 * GW] = 1.0       # lane0 one-hot START
    consts = consts.astype(ml_dtypes.bfloat16)
    evconsts = evconsts.astype(ml_dtypes.bfloat16)

    shared = {"consts": consts, "evconsts": evconsts}
    return chunks, shared


def _postprocess(results, transitions):
    """Combine per-core sum banks into logZ [B] (float64 host math)."""
    out = np.empty((NCORE, 2, GW), dtype=np.float64)

    def rowcol(s, b, base):
        c, l, q = _slot_of(s, b)
        h, cb = q // 4, q % 4
        return 16 * c + base + 2 * h + l, cb * GW

    for core in range(NCORE):
        sm = np.asarray(results[core]["sums"], dtype=np.float64)
        acc = np.zeros((2, GW), dtype=np.float64)
        for s in range(S - 1):
            ebase = 8 if (s == 0 and RSTARS[0] != R - 1) else 4
            for b in range(2):
                er, ec = rowcol(s, b, ebase)
                mr, mc = rowcol(s + 1, b, 0)
                acc[b] += (np.log(sm[er, ec:ec + GW])
                           - np.log(sm[mr, mc:mc + GW]))
        for b in range(2):
            tr, tc = rowcol(S - 1, b, 8)
            acc[b] += np.log(sm[tr, tc:tc + GW])
        out[core] = acc - T * BIAS_C
    return out.reshape(B).astype(np.float32)


def kernel(feats, transitions):
    from concourse.bass_utils import run_bass_kernel_spmd

    chunks, shared = _pack_host(feats, transitions)
    if "nc" not in _cache:
        _cache["nc"] = _build()
    nc = _cache["nc"]

    in_maps = [
        dict(shared, **{f"feats{q}": chunks[q][c] for q in range(len(CHUNKS))})
        for c in range(NCORE)
    ]
    res = run_bass_kernel_spmd(nc, in_maps, list(range(NCORE)))
    return _postprocess(res.results, transitions)


# revision 38
# speedup vs baseline: 1.2975x; 1.2975x over previous
"""Trainium2 Bass kernel for batched CRF forward algorithm (log-partition).

Reference: alpha_t[b,i] = logsumexp_j(alpha_{t-1}[b,j] + trans[i,j]) + feat_t[b,i]
           logZ[b] = logsumexp_i(alpha_{T-1}[b,i] + trans[STOP,i])

Exp-domain device recursion per step: X' = (W @ X) * F with
F = exp(feat + BIAS_C) precomputed on host (bf16).

Time is cut into S=24 forward segments run in lockstep (Birkhoff
contraction: a 1-step warm-up from a ones vector leaves ~e-4 direction
error; unknown scales cancel in telescoped bridge corrections computed
from per-slot column sums, all evaluated on-device by tiny ones/wstop
matmuls into one PSUM bank -- the only kernel output).

Per core: 3 chains x [96, 1024] state tiles (2 lanes of 48 tags x 8
column groups of 128 seqs = 16 slots/chain), R=12 rounds. Per round per
chain: 2 matmuls (one per PSUM bank), then the PSUM->SBUF move is split
to balance engines: DVE does a direct 1x TT on cols [0:A), ACT copies
cols [A:1024) to SBUF bf16 where DVE re-TTs them at 2x mode. Measured
steady-state: DVE ~= ACT ~= 790 ns per chain-step -> ~2.37 us/round.

Sharding: B=2048 over 8 cores (data parallel), no collectives.
"""

import numpy as np

B, T, K = 2048, 256, 48
NCORE = 8
START, STOP = 46, 47
BIAS_C = -4.33

C = 3                    # chains per core
PP = 2 * K               # 96 partitions (2 lanes)
GW = 128                 # slot (group) width in columns
NG = 8                   # column groups per chain
CT = NG * GW             # 1024 columns per chain tile
R = 12                   # rounds
S = 24                   # segments (= C * NG * 2 lanes / 2 blocks)
WARM = 1                 # warm-up rounds per non-exact segment
ASPL = 216               # DVE-direct columns; ACT copies the rest
NTILE = R * C            # 36 F tiles consumed in (r, c) order
CHUNKS = [1] * 12 + [2] * 12   # F tiles per DMA chunk
NWARM = 3                # dummy matmuls to release the PE HAM clock gate
NLHS = 7                 # sum-event lhsT pairs: M x3, S0, F x3
CW = PP + 2 * GW + NLHS * 2 * 48  # consts: W2 | seg0 init | event lhsTs

# segment schedule (mirrors sim.py): junction deltas sum to T - R
_deltas = [R - WARM] * (S - 1)
_exc = (R - WARM) * (S - 1) - (T - R)
_i = 0
while _exc > 0:
    _t = min(_exc, R - WARM - 1)
    _deltas[_i] -= _t
    _exc -= _t
    _i += 1
STARTS = np.concatenate([[0], np.cumsum(_deltas)]).astype(int)
RSTARS = [d + WARM - 1 for d in _deltas]     # E_s snapshot round per junction
EVROUNDS = sorted(set(RSTARS))               # extra E-event rounds (2 and 11)
assert STARTS[-1] == T - R and sum(CHUNKS) == NTILE

_cache = {}


def _seg_of(c, l, q):
    return 8 * c + 4 * l + q // 2


def _slot_of(s, b):
    """segment s, block b -> (chain, lane, group)."""
    return s // 8, (s % 8) // 4, 2 * (s % 4) + b


def _build():
    import concourse.bass as bass
    import concourse.bacc as bacc
    import concourse.mybir as mybir
    from concourse import tile

    bf16 = mybir.dt.bfloat16
    f32 = mybir.dt.float32
    PSUM = bass.MemorySpace.PSUM

    nc = bacc.Bacc(None, target_bir_lowering=False)

    fdr = [nc.dram_tensor(f"feats{q}", [PP, n * CT], bf16,
                          kind="ExternalInput") for q, n in enumerate(CHUNKS)]
    cdr = nc.dram_tensor("consts", [PP, CW], bf16, kind="ExternalInput")
    odr = nc.dram_tensor("sums", [48, 512], f32, kind="ExternalOutput")

    with tile.TileContext(nc) as tc:
        with (
            tc.tile_pool(name="const", bufs=1) as cpool,
            tc.tile_pool(name="fchunk", bufs=1) as fpool,
            tc.tile_pool(name="state", bufs=2) as spool,
            tc.tile_pool(name="ycp", bufs=1) as ypool,
            tc.tile_pool(name="ps", bufs=1, space=PSUM) as pspool,
        ):
            csb = cpool.tile([PP, CW], bf16, name="consts", tag="consts")
            isb = cpool.tile([PP, C * CT], bf16, name="inits", tag="inits")
            osb = cpool.tile([48, 512], f32, name="osums", tag="osums")
            w2 = csb[:, 0:PP]

            def evlhs(pair, h):
                off = PP + 2 * GW + (2 * pair + h) * 48
                return csb[:, off:off + 48]

            # warm-up scratch: memset is fast and has no DMA dependency, so
            # the PE burst can start right after the runtime preamble
            wt = cpool.tile([PP, 512], bf16, name="wtile", tag="wtile")
            nc.gpsimd.memset(wt[:], 1.0)

            # dummy activation: pulls the ~1.3us ACT_TABLE_LOAD out of
            # the main loop (first scalar.copy otherwise pays it there)
            nc.scalar.copy(wt[0:1, 0:8], wt[0:1, 8:16])

            nc.sync.dma_start(csb[:], cdr[:])
            # inits built on-device: ones everywhere, one-hot START for
            # segment 0 (chain 0, lane 0, groups 0..1) copied from consts
            nc.gpsimd.memset(isb[:], 1.0)
            nc.vector.tensor_copy(isb[:, 0:2 * GW], csb[:, PP:PP + 2 * GW])

            fts = []
            cstart = np.cumsum([0] + CHUNKS)
            for q, n in enumerate(CHUNKS):
                ft = fpool.tile([PP, n * CT], bf16, name=f"f{q}", tag=f"f{q}")
                # all on the sync HWDGE ring: one queue sustains the full
                # DMA rate, and FIFO order matches consumption order.
                # (A second ring steals little extra bandwidth, and issues
                # on the scalar ring stall the ACT sequencer mid-loop.)
                nc.sync.dma_start(ft[:], fdr[q][:])
                fts.append(ft)

            def ftile(r, c):
                m = r * C + c
                q = int(np.searchsorted(cstart, m, side="right")) - 1
                off = (m - int(cstart[q])) * CT
                return fts[q][:, off:off + CT]

            psum = [pspool.tile([PP, CT], f32, name=f"p{c}", tag=f"p{c}")
                    for c in range(C)]
            sums = pspool.tile([48, 512], f32, name="sums", tag="sums")
            scr = pspool.tile([PP, 512], f32, name="scr", tag="scr")

            # back-to-back dummy matmuls during the DMA head keep the PE
            # HAM activity window busy so the clock gate opens (1.2->2.4
            # GHz) before the main loop starts
            for _ in range(NWARM):
                nc.tensor.matmul(scr[:], wt[:, 0:PP], wt[:],
                                 start=True, stop=True)
            ys = [ypool.tile([PP, CT - ASPL], bf16, name=f"y{c}", tag=f"y{c}")
                  for c in range(C)]

            xs = [isb[:, c * CT:(c + 1) * CT] for c in range(C)]

            # PE fillers: the HAM clock gate re-throttles on sub-us PE idle
            # gaps, so keep PE near-continuously busy in every round
            # (fillers run in otherwise-idle PE windows; in-order issue
            # delays a ready real matmul by at most one filler)
            def filler(n):
                for _ in range(n):
                    nc.tensor.matmul(scr[:], wt[:, 0:PP], wt[:],
                                     start=True, stop=True)

            for r in range(R):
                xn = [spool.tile([PP, CT], bf16, name=f"x{c}", tag=f"x{c}")[:]
                      for c in range(C)]
                for c in range(C):
                    nc.tensor.matmul(psum[c][:, 0:512], w2, xs[c][:, 0:512],
                                     start=True, stop=True)
                    nc.tensor.matmul(psum[c][:, 512:CT], w2, xs[c][:, 512:CT],
                                     start=True, stop=True)
                    filler(1 if c < 2 else 2)
                if r <= 1:
                    filler(2)
                if r == R - 1:
                    filler(3)   # bridge PE into the tail event matmuls
                # direct TTs first so DVE is not stalled behind the copies
                for c in range(C):
                    f = ftile(r, c)
                    nc.vector.tensor_mul(xn[c][:, 0:ASPL],
                                         psum[c][:, 0:ASPL], f[:, 0:ASPL])
                for c in range(C):
                    nc.scalar.copy(ys[c][:], psum[c][:, ASPL:CT])
                for c in range(C):
                    f = ftile(r, c)
                    nc.vector.tensor_mul(xn[c][:, ASPL:CT],
                                         ys[c][:], f[:, ASPL:CT])
                xs = xn

                # bridge sums: one accumulation group into sums[48,512].
                # Row selection is encoded in sparse [96,48] lhsTs:
                # rows 16c+{0:4}=M(r=0), {4:8}=E(r=R-1), {8:12}=E(r=2) on
                # chain 0 / wstop(r=R-1) on chain 2.  lhsT pairs: M_c=0..2,
                # S0=3 (chain0 r=2), F_c=4..6 (E final, +wstop on chain 2).
                for c in range(C):
                    evs = []
                    if r == WARM - 1:
                        evs.append(c)               # M_c
                    if r == RSTARS[0] and c == 0 and RSTARS[0] != R - 1:
                        evs.append(3)               # S0
                    if r == R - 1:
                        evs.append(4 + c)           # F_c
                    if r == R - 1:
                        filler(1)
                    for pair in evs:
                        for h in range(2):
                            first = (r == WARM - 1 and c == 0 and h == 0)
                            last = (r == R - 1 and c == C - 1 and h == 1)
                            nc.tensor.matmul(
                                sums[:, :], evlhs(pair, h),
                                xs[c][:, 512 * h:512 * (h + 1)],
                                start=first, stop=last,
                                skip_group_check=True)

            nc.scalar.copy(osb[:], sums[:])
            nc.sync.dma_start(odr[:], osb[:])

    nc.compile()
    return nc


def _pack_host(feats, transitions):
    """Host-side sharding/layout prep (numpy only)."""
    import ml_dtypes

    feats = np.asarray(feats, dtype=np.float32)
    trans = np.asarray(transitions, dtype=np.float64)

    F = np.exp(feats + BIAS_C).reshape(NCORE, 2, GW, T, K)

    # F tiles: arr[core, part, r, c, q, col]
    arr = np.empty((NCORE, PP, R, C, NG, GW), dtype=np.float32)
    rr = np.arange(R)
    for c in range(C):
        for l in range(2):
            for q in range(NG):
                s = _seg_of(c, l, q)
                b = q % 2
                ts = STARTS[s] + rr
                # F[:, b, :, ts, :] -> [core, col, R, K] -> [core, K, R, col]
                blk = F[:, b][:, :, ts, :].transpose(0, 3, 2, 1)
                arr[:, 48 * l:48 * (l + 1), :, c, q, :] = blk
    flat = np.ascontiguousarray(
        arr.reshape(NCORE, PP, NTILE * CT)).astype(ml_dtypes.bfloat16)
    cstart = np.cumsum([0] + CHUNKS)
    chunks = [np.ascontiguousarray(flat[:, :, cstart[q] * CT:cstart[q + 1] * CT])
              for q in range(len(CHUNKS))]

    W = np.exp(trans)
    consts = np.zeros((PP, CW), dtype=np.float64)
    consts[:K, :K] = W.T
    consts[K:, K:PP] = W.T
    wstop = np.exp(trans[STOP])
    # event lhsTs [96,48]: column j routes a lane-sum to output row j
    for pair in range(NLHS):
        for h in range(2):
            L = np.zeros((PP, 48), dtype=np.float64)
            if pair < 3:                      # M_c: rows 16c+2h+l
                c, base = pair, 0
            elif pair == 3:                   # S0: chain0 r=2, rows 8+2h+l
                c, base = 0, 8
            else:                             # F_c: rows 16c+4+2h+l
                c, base = pair - 4, 4
            for l in range(2):
                L[48 * l:48 * (l + 1), 16 * c + base + 2 * h + l] = 1.0
            if pair == 6:                     # chain2 final: + wstop rows
                L[K:, 16 * 2 + 8 + 2 * h + 1] = wstop
            consts[:, PP + 2 * GW + (2 * pair + h) * 48:
                   PP + 2 * GW + (2 * pair + h + 1) * 48] = L
    consts[K:, PP:PP + 2 * GW] = 1.0          # seg0 init block: lane1 ones
    consts[START, PP:PP + 2 * GW] = 1.0       # lane0 one-hot START
    consts = consts.astype(ml_dtypes.bfloat16)

    shared = {"consts": consts}
    return chunks, shared


def _postprocess(results, transitions):
    """Combine per-core sum banks into logZ [B] (float64 host math)."""
    out = np.empty((NCORE, 2, GW), dtype=np.float64)

    def rowcol(s, b, base):
        c, l, q = _slot_of(s, b)
        h, cb = q // 4, q % 4
        return 16 * c + base + 2 * h + l, cb * GW

    for core in range(NCORE):
        sm = np.asarray(results[core]["sums"], dtype=np.float64)
        acc = np.zeros((2, GW), dtype=np.float64)
        for s in range(S - 1):
            ebase = 8 if (s == 0 and RSTARS[0] != R - 1) else 4
            for b in range(2):
                er, ec = rowcol(s, b, ebase)
                mr, mc = rowcol(s + 1, b, 0)
                acc[b] += (np.log(sm[er, ec:ec + GW])
                           - np.log(sm[mr, mc:mc + GW]))
        for b in range(2):
            tr, tc = rowcol(S - 1, b, 8)
            acc[b] += np.log(sm[tr, tc:tc + GW])
        out[core] = acc - T * BIAS_C
    return out.reshape(B).astype(np.float32)


def kernel(feats, transitions):
    from concourse.bass_utils import run_bass_kernel_spmd

    chunks, shared = _pack_host(feats, transitions)
    if "nc" not in _cache:
        _cache["nc"] = _build()
    nc = _cache["nc"]

    in_maps = [
        dict(shared, **{f"feats{q}": chunks[q][c] for q in range(len(CHUNKS))})
        for c in range(NCORE)
    ]
    res = run_bass_kernel_spmd(nc, in_maps, list(range(NCORE)))
    return _postprocess(res.results, transitions)
